# revision 1
# baseline (speedup 1.0000x reference)
"""2D DCT-II (4096x4096) on 8 Trainium2 NeuronCores (axon/PJRT SPMD).

Math: the reference computes C = G_M @ y @ G_N^T, y = x[pm][:, pn] (Makhoul
even-odd reorder), G built from the DFT kernel and the expk inputs:
  G_M[u,i] = 0.5*(eMr[u]*cos(2pi*u*i/M) + eMi[u]*sin(2pi*u*i/M))
  G_N[v,j] = 2.0*(eNr[v]*cos(2pi*v*j/N) + eNi[v]*sin(2pi*v*j/N))
Permutations fold into the tables (A[:, pm[i]] = G[:, i]), so on device:
  C = A_M @ x @ A_N^T        -- two dense 4096^3 matmuls.

Distribution (8 cores):
  phase 1: core k computes W_k = x[rows_k, :] @ A_N^T  (rows_k = 512k..+512),
           written in 8 column blocks [8, 512, 512] for the exchange.
  AllToAll: block (k -> j): W_k[:, cols_j]; after, core k holds
           W[:, cols_k] as [8, 512, 512] (m-th block = rows 512m).
  phase 2: core k computes C[:, cols_k] = A_M @ W[:, cols_k].
Host: builds A tables from expk (cached), slices x^T, concatenates shards.
Matmuls run as float32r (TF32-like, ~1e-4 rel err) via AP bitcast.
"""
import numpy as np

_NCORES = 8
_SZ = 4096
_RPC = _SZ // _NCORES  # 512 rows/cols per core
_KT = _SZ // 128       # 32 contraction tiles

_state = {}


# --------------------------------------------------------------------------
# Bass kernel
# --------------------------------------------------------------------------
def _build_bass(sz=_SZ):
    import concourse.bacc as bacc
    import concourse.mybir as mybir
    from concourse.tile import TileContext

    fp32 = mybir.dt.float32
    fp32r = mybir.dt.float32r
    _SZ = sz
    _RPC = _SZ // _NCORES
    _KT = _SZ // 128
    nc = bacc.Bacc("TRN2", target_bir_lowering=False, debug=False,
                   num_devices=_NCORES)
    xT = nc.declare_dram_parameter("xT", [_SZ, _RPC], fp32r, isOutput=False)
    annT = nc.declare_dram_parameter("annT", [_SZ, _SZ], fp32r, isOutput=False)
    amT = nc.declare_dram_parameter("amT", [_SZ, _SZ], fp32r, isOutput=False)
    cout = nc.declare_dram_parameter("cout", [_SZ, _RPC], fp32, isOutput=True)

    w_send = nc.dram_tensor("w_send", [_NCORES, _RPC, _RPC], fp32r)
    w_recv = nc.dram_tensor("w_recv", [_NCORES, _RPC, _RPC], fp32r)

    with TileContext(nc) as tc:
        # ---------- phase 1: W_k = x_k @ A_N^T ----------
        # xT resident in SBUF as [128, (kt, r)] : xT[kt*128+p, r]
        # annT streamed in 256-wide v panels [128, (kt, v)]
        with (
            tc.tile_pool(name="xw", bufs=1) as xw_pool,
            tc.tile_pool(name="an", bufs=3) as an_pool,
            tc.tile_pool(name="ps1", bufs=8, space="PSUM") as ps1_pool,
            tc.tile_pool(name="ev1", bufs=4) as ev1_pool,
        ):
            xw = xw_pool.tile([128, _KT * _RPC], fp32r)  # 8 MB
            nc.sync.dma_start(
                out=xw[:].rearrange("p (kt r) -> p kt r", kt=_KT),
                in_=xT[:].rearrange("(kt p) r -> p kt r", p=128))
            VP = min(256, _RPC)
            for vp in range(_SZ // VP):
                an = an_pool.tile([128, _KT * VP], fp32r, tag="an")  # 4 MB
                nc.sync.dma_start(
                    out=an[:].rearrange("p (kt v) -> p kt v", kt=_KT),
                    in_=annT[:, vp * VP:(vp + 1) * VP]
                    .rearrange("(kt p) v -> p kt v", p=128))
                for rt in range(_RPC // 128):
                    ps = ps1_pool.tile([128, VP], fp32, tag="ps")
                    for k in range(_KT):
                        nc.tensor.matmul(
                            ps[:],
                            xw[:, k * _RPC + rt * 128:
                                  k * _RPC + rt * 128 + 128],
                            an[:, k * VP:(k + 1) * VP],
                            start=(k == 0), stop=(k == _KT - 1))
                    ev = ev1_pool.tile([128, VP], fp32r, tag="ev")
                    nc.vector.tensor_copy(ev[:], ps[:])
                    # rows rt*128..+128 of W_k, cols vp*VP..+VP ->
                    # w_send[block j, r, c] with j = col//512
                    j = (vp * VP) // _RPC
                    c0 = (vp * VP) % _RPC
                    nc.sync.dma_start(
                        out=w_send[j, rt * 128:(rt + 1) * 128, c0:c0 + VP],
                        in_=ev[:])

        # ---------- exchange ----------
        nc.gpsimd.collective_compute(
            "AllToAll",
            mybir.AluOpType.bypass,
            ins=[w_send[:]],
            outs=[w_recv[:]],
            replica_groups=[list(range(_NCORES))],
        )

        # ---------- phase 2: C[:, cols_k] = A_M @ W[:, cols_k] ----------
        # w_recv resident [128, (kt, v)]: global row i = kt*128 + p
        #   w_recv[m, i2, v], m = kt//4, i2 = (kt%4)*128 + p
        # amT streamed per u-tile [128, (kt, u)]
        with (
            tc.tile_pool(name="wr", bufs=1) as wr_pool,
            tc.tile_pool(name="am", bufs=3) as am_pool,
            tc.tile_pool(name="ps2", bufs=8, space="PSUM") as ps2_pool,
            tc.tile_pool(name="ev2", bufs=4) as ev2_pool,
        ):
            wr = wr_pool.tile([128, _KT * _RPC], fp32r)  # 8 MB
            nc.sync.dma_start(
                out=wr[:].rearrange("p (m s v) -> p m s v", m=_NCORES, s=4),
                in_=w_recv[:].rearrange("m (s p) v -> p m s v", p=128))
            for ut in range(_SZ // 128):
                am = am_pool.tile([128, _KT * 128], fp32r, tag="am")  # 2 MB
                nc.sync.dma_start(
                    out=am[:].rearrange("p (kt u) -> p kt u", kt=_KT),
                    in_=amT[:, ut * 128:(ut + 1) * 128]
                    .rearrange("(kt p) u -> p kt u", p=128))
                VP2 = min(256, _RPC)
                for half in range(_RPC // VP2):
                    ps = ps2_pool.tile([128, VP2], fp32, tag="ps2")
                    for k in range(_KT):
                        nc.tensor.matmul(
                            ps[:],
                            am[:, k * 128:(k + 1) * 128],
                            wr[:, k * _RPC + half * VP2:
                                  k * _RPC + (half + 1) * VP2],
                            start=(k == 0), stop=(k == _KT - 1))
                    ev = ev2_pool.tile([128, VP2], fp32, tag="ev2")
                    nc.vector.tensor_copy(ev[:], ps[:])
                    nc.sync.dma_start(
                        out=cout[ut * 128:(ut + 1) * 128,
                                 half * VP2:(half + 1) * VP2],
                        in_=ev[:])

    nc.compile()
    return nc


# --------------------------------------------------------------------------
# PJRT SPMD runner (compile once, run many)
# --------------------------------------------------------------------------
def _build_runner(nc, n_cores):
    import jax
    from jax.sharding import Mesh, PartitionSpec
    from jax.experimental.shard_map import shard_map
    import concourse.mybir as mybir
    from concourse import bass2jax
    from concourse.bass2jax import _bass_exec_p, partition_id_tensor

    bass2jax.install_neuronx_cc_hook()
    partition_name = (nc.partition_id_tensor.name
                      if nc.partition_id_tensor else None)

    in_names, out_names, out_avals, zero_outs = [], [], [], []
    for alloc in nc.m.functions[0].allocations:
        if not isinstance(alloc, mybir.MemoryLocationSet):
            continue
        name = alloc.memorylocations[0].name
        if alloc.kind == "ExternalInput":
            if name != partition_name:
                in_names.append(name)
        elif alloc.kind == "ExternalOutput":
            shape = tuple(alloc.tensor_shape)
            dtype = mybir.dt.np(alloc.dtype)
            out_names.append(name)
            out_avals.append(jax.core.ShapedArray(shape, dtype))
            zero_outs.append(np.zeros(shape, dtype))
    n_params = len(in_names)
    n_outs = len(out_avals)
    in_names_all = list(in_names) + out_names
    if partition_name is not None:
        in_names_all = in_names_all + [partition_name]
    donate = tuple(range(n_params, n_params + n_outs))

    def _body(*args):
        operands = list(args)
        if partition_name is not None:
            operands.append(partition_id_tensor())
        outs = _bass_exec_p.bind(
            *operands,
            out_avals=tuple(out_avals),
            in_names=tuple(in_names_all),
            out_names=tuple(out_names),
            lowering_input_output_aliases=(),
            sim_require_finite=True,
            sim_require_nnan=True,
            nc=nc,
        )
        return tuple(outs)

    devices = jax.devices()[:n_cores]
    mesh = Mesh(np.asarray(devices), ("core",))
    sharded = jax.jit(
        shard_map(_body, mesh=mesh,
                  in_specs=(PartitionSpec("core"),) * (n_params + n_outs),
                  out_specs=(PartitionSpec("core"),) * n_outs,
                  check_rep=False),
        donate_argnums=donate, keep_unused=True)

    from jax.sharding import NamedSharding
    shard = NamedSharding(mesh, PartitionSpec("core"))
    _dev_cache = {}

    import jax.numpy as jnp
    _zero_shapes = [(n_cores * z.shape[0], *z.shape[1:]) for z in zero_outs]
    _zero_dtypes = [z.dtype for z in zero_outs]

    _make_zeros = jax.jit(
        lambda: tuple(jnp.zeros(s, d)
                      for s, d in zip(_zero_shapes, _zero_dtypes)),
        out_shardings=(shard,) * len(_zero_shapes))

    def run(in_maps, cache_names=(), fetch=True):
        concat_in = []
        for i, name in enumerate(in_names):
            if name in cache_names and name in _dev_cache:
                concat_in.append(_dev_cache[name])
                continue
            arr = np.concatenate(
                [np.asarray(in_maps[c][name]) for c in range(n_cores)], axis=0)
            arr = jax.device_put(arr, shard)
            if name in cache_names:
                jax.block_until_ready(arr)
                _dev_cache[name] = arr
            concat_in.append(arr)
        concat_zeros = _make_zeros()
        raw = sharded(*concat_in, *concat_zeros)
        if not fetch:
            import jax as _jax
            _jax.block_until_ready(raw)
            return raw
        out_arrs = [np.asarray(o) for o in raw]
        return [
            {name: out_arrs[i].reshape(n_cores, *out_avals[i].shape)[c]
             for i, name in enumerate(out_names)}
            for c in range(n_cores)]

    run.dev_cache = _dev_cache
    return run


# --------------------------------------------------------------------------
# host-side tables
# --------------------------------------------------------------------------
def _tables(expkM, expkN):
    key = (expkM.tobytes(), expkN.tobytes())
    cached = _state.get("tables")
    if cached is not None and cached[0] == key:
        return cached[1], cached[2]
    run = _state.get("run")
    if run is not None:
        run.dev_cache.clear()
    n = _SZ
    i = np.arange(n)
    pm = np.where(i < (n + 1) // 2, 2 * i, 2 * (n - i) - 1)
    pinv = np.empty(n, dtype=np.int64)
    pinv[pm] = i
    # Cp[j, v] = cos(2pi * pinv[j] * v / n); note cos/sin tables are symmetric
    ang = (2.0 * np.pi / n) * np.outer(pinv.astype(np.float64),
                                       i.astype(np.float64))
    Cp = np.cos(ang)
    Sp = np.sin(ang)
    eMr = expkM[:, 0].astype(np.float64)
    eMi = expkM[:, 1].astype(np.float64)
    eNr = expkN[:, 0].astype(np.float64)
    eNi = expkN[:, 1].astype(np.float64)
    annT = np.ascontiguousarray(
        (2.0 * (Cp * eNr[None, :] + Sp * eNi[None, :])).astype(np.float32))
    amT = np.ascontiguousarray(
        (0.5 * (Cp * eMr[None, :] + Sp * eMi[None, :])).astype(np.float32))
    _state["tables"] = (key, annT, amT)
    return annT, amT


def kernel(x, expkM, expkN, M, N):
    x = np.asarray(x, dtype=np.float32)
    expkM = np.asarray(expkM, dtype=np.float32)
    expkN = np.asarray(expkN, dtype=np.float32)
    assert x.shape == (_SZ, _SZ)

    annT, amT = _tables(expkM, expkN)
    if "run" not in _state:
        _state["run"] = _build_runner(_build_bass(), _NCORES)
    run = _state["run"]

    xT = np.ascontiguousarray(x.T)
    in_maps = [
        {"xT": np.ascontiguousarray(xT[:, k * _RPC:(k + 1) * _RPC]),
         "annT": annT, "amT": amT}
        for k in range(_NCORES)]
    outs = run(in_maps, cache_names=("annT", "amT"))
    C = np.concatenate([outs[k]["cout"] for k in range(_NCORES)], axis=1)
    return np.ascontiguousarray(C, dtype=np.float32)



# revision 4
# speedup vs baseline: 1011.6739x; 1011.6739x over previous
"""2D DCT-II (4096x4096) on 8 Trainium2 NeuronCores (axon/PJRT SPMD).

Math: the reference computes C = G_M @ y @ G_N^T, y = x[pm][:, pn] (Makhoul
even-odd reorder). Folding the permutations and expk twiddles into dense
tables gives C = A_M @ x @ A_N^T, where
  annT[j, v] = A_N[v, j] = 2*cos(pi*v*(4*pinv[j]+1)/(2N))
  amT[g, u]  = A_M[u, g] = 0.5*cos(pi*u*(4*pinv[g]+1)/(2M))
(computed host-side from expk via the int-mod angle trick, stored bf16).

Device schedule (core k of 8, all matmuls bf16 into fp32 PSUM):
  phase 0: DMA x[rows_k] (512,4096) fp32, convert bf16, TensorE-transpose
           128x128 tiles into xw[c-part, (kt, r)].
  phase 1: W_k = x[rows_k] @ A_N^T, streamed over 8 v-panels of 512;
           PSUM evac to bf16 -> w_send[j=vp, r, c].
  AllToAll #1 (bf16): core k ends with W[:, cols_k] blocks.
  phase 2: C[:, cols_k] = A_M @ W[:, cols_k], amT streamed; evac bf16 ->
           c_send[j=ut//4, r, c].
  AllToAll #2 (bf16): core k ends with C[rows_k, cols_m] blocks.
  final:   convert bf16->fp32, assemble cout = C[rows_k, :].
Host does no transposes/concats: x is device_put row-sharded as-is and the
shard_map output IS the full (4096,4096) C. Tables are device-cached.
"""
import numpy as np

_NCORES = 8
_SZ = 4096
_RPC = _SZ // _NCORES  # 512 rows/cols per core
_KT = _SZ // 128       # 32 contraction tiles

_state = {}


# --------------------------------------------------------------------------
# Bass kernel
# --------------------------------------------------------------------------
def _build_bass():
    import concourse.bacc as bacc
    import concourse.mybir as mybir
    from concourse.tile import TileContext
    from concourse import masks

    fp32 = mybir.dt.float32
    bf16 = mybir.dt.bfloat16
    nc = bacc.Bacc("TRN2", target_bir_lowering=False, debug=False,
                   num_devices=_NCORES)
    x_in = nc.declare_dram_parameter("x_in", [_RPC, _SZ], fp32,
                                     isOutput=False)
    annT = nc.declare_dram_parameter("annT_b", [_SZ, _SZ], bf16,
                                     isOutput=False)
    amT = nc.declare_dram_parameter("amT_b", [_SZ, _SZ], bf16,
                                    isOutput=False)
    cout = nc.declare_dram_parameter("cout", [_RPC, _SZ], fp32,
                                     isOutput=True)

    w_send = nc.dram_tensor("w_send", [_NCORES, _RPC, _RPC], bf16)
    w_recv = nc.dram_tensor("w_recv", [_NCORES, _RPC, _RPC], bf16)
    c_send = nc.dram_tensor("c_send", [_NCORES, _RPC, _RPC], bf16)
    c_recv = nc.dram_tensor("c_recv", [_NCORES, _RPC, _RPC], bf16)

    RT = _RPC // 128  # 4 row tiles per core

    with TileContext(nc) as tc:
        with (
            tc.tile_pool(name="ident", bufs=1) as ident_pool,
            tc.tile_pool(name="xw", bufs=1) as xw_pool,
        ):
            ident = ident_pool.tile([128, 128], bf16)
            masks.make_identity(nc, ident[:])
            # xw[c_part, (kt, r)] : x[rows_k]^T in bf16, c = kt*128+p
            xw = xw_pool.tile([128, _KT * _RPC], bf16)  # 32 KB/part

            # ---------- phase 0: load + transpose x ----------
            with (
                tc.tile_pool(name="xf", bufs=1) as xf_pool,
                tc.tile_pool(name="xb", bufs=1) as xb_pool,
                tc.tile_pool(name="pst", bufs=4, space="PSUM") as pst_pool,
            ):
                xf = xf_pool.tile([128, RT * _SZ], fp32)  # 64 KB/part
                nc.sync.dma_start(
                    out=xf[:].rearrange("p (rt c) -> p rt c", rt=RT),
                    in_=x_in[:].rearrange("(rt p) c -> p rt c", p=128))
                xb = xb_pool.tile([128, RT * _SZ], bf16)  # 32 KB/part
                for rt in range(RT):
                    nc.vector.tensor_copy(
                        xb[:, rt * _SZ:(rt + 1) * _SZ],
                        xf[:, rt * _SZ:(rt + 1) * _SZ])
                for ct in range(_KT):
                    ps = pst_pool.tile([128, _RPC], bf16, tag="pst")
                    for rt in range(RT):
                        nc.tensor.transpose(
                            ps[:, rt * 128:(rt + 1) * 128],
                            xb[:, rt * _SZ + ct * 128:
                                  rt * _SZ + ct * 128 + 128],
                            ident[:])
                    nc.vector.tensor_copy(
                        xw[:, ct * _RPC:(ct + 1) * _RPC], ps[:])

            # ---------- phase 1: W_k = x_k @ A_N^T ----------
            with (
                tc.tile_pool(name="an", bufs=3) as an_pool,
                tc.tile_pool(name="ps1", bufs=8, space="PSUM") as ps1_pool,
                tc.tile_pool(name="ev1", bufs=4) as ev1_pool,
            ):
                for vp in range(_NCORES):  # 512-wide v panel = dest core
                    an = an_pool.tile([128, _KT * _RPC], bf16, tag="an")
                    nc.sync.dma_start(
                        out=an[:].rearrange("p (kt v) -> p kt v", kt=_KT),
                        in_=annT[:, vp * _RPC:(vp + 1) * _RPC]
                        .rearrange("(kt p) v -> p kt v", p=128))
                    for rt in range(RT):
                        ps = ps1_pool.tile([128, _RPC], fp32, tag="ps")
                        for k in range(_KT):
                            nc.tensor.matmul(
                                ps[:],
                                xw[:, k * _RPC + rt * 128:
                                      k * _RPC + rt * 128 + 128],
                                an[:, k * _RPC:(k + 1) * _RPC],
                                start=(k == 0), stop=(k == _KT - 1))
                        ev = ev1_pool.tile([128, _RPC], bf16, tag="ev")
                        nc.vector.tensor_copy(ev[:], ps[:])
                        nc.sync.dma_start(
                            out=w_send[vp, rt * 128:(rt + 1) * 128, :],
                            in_=ev[:])

        # ---------- exchange 1 ----------
        nc.gpsimd.collective_compute(
            "AllToAll",
            mybir.AluOpType.bypass,
            ins=[w_send[:]],
            outs=[w_recv[:]],
            replica_groups=[list(range(_NCORES))],
        )

        # ---------- phase 2: C[:, cols_k] = A_M @ W[:, cols_k] ----------
        # w_recv[m, r, v]: global row g = m*512 + r = kt*128 + p
        with (
            tc.tile_pool(name="wr", bufs=1) as wr_pool,
            tc.tile_pool(name="am", bufs=3) as am_pool,
            tc.tile_pool(name="ps2", bufs=8, space="PSUM") as ps2_pool,
            tc.tile_pool(name="ev2", bufs=4) as ev2_pool,
        ):
            wr = wr_pool.tile([128, _KT * _RPC], bf16)  # 32 KB/part
            nc.sync.dma_start(
                out=wr[:].rearrange("p (m s v) -> p m s v", m=_NCORES, s=4),
                in_=w_recv[:].rearrange("m (s p) v -> p m s v", p=128))
            for up in range(_SZ // 256):  # 256-wide u panels (2 ut each)
                am = am_pool.tile([128, _KT * 256], bf16, tag="am")
                nc.sync.dma_start(
                    out=am[:].rearrange("p (kt u) -> p kt u", kt=_KT),
                    in_=amT[:, up * 256:(up + 1) * 256]
                    .rearrange("(kt p) u -> p kt u", p=128))
                for half in range(2):
                    ut = up * 2 + half
                    ps = ps2_pool.tile([128, _RPC], fp32, tag="ps2")
                    for k in range(_KT):
                        nc.tensor.matmul(
                            ps[:],
                            am[:, k * 256 + half * 128:
                                  k * 256 + half * 128 + 128],
                            wr[:, k * _RPC:(k + 1) * _RPC],
                            start=(k == 0), stop=(k == _KT - 1))
                    ev = ev2_pool.tile([128, _RPC], bf16, tag="ev2")
                    nc.vector.tensor_copy(ev[:], ps[:])
                    nc.sync.dma_start(
                        out=c_send[ut // RT, (ut % RT) * 128:
                                   (ut % RT) * 128 + 128, :],
                        in_=ev[:])

        # ---------- exchange 2: reshard C cols -> rows ----------
        nc.gpsimd.collective_compute(
            "AllToAll",
            mybir.AluOpType.bypass,
            ins=[c_send[:]],
            outs=[c_recv[:]],
            replica_groups=[list(range(_NCORES))],
        )

        # ---------- final: assemble C[rows_k, :] in fp32 ----------
        with (
            tc.tile_pool(name="cr", bufs=2) as cr_pool,
            tc.tile_pool(name="co", bufs=2) as co_pool,
        ):
            for rt in range(RT):
                cr = cr_pool.tile([128, _NCORES * _RPC], bf16, tag="cr")
                nc.sync.dma_start(
                    out=cr[:].rearrange("p (m c) -> p m c", m=_NCORES),
                    in_=c_recv[:, rt * 128:(rt + 1) * 128, :]
                    .rearrange("m p c -> p m c"))
                co = co_pool.tile([128, _SZ], fp32, tag="co")
                nc.vector.tensor_copy(co[:], cr[:])
                nc.sync.dma_start(
                    out=cout[rt * 128:(rt + 1) * 128, :], in_=co[:])

    nc.compile()
    return nc


# --------------------------------------------------------------------------
# PJRT SPMD runner (compile once, run many; single fused jit per call)
# --------------------------------------------------------------------------
def _build_runner(nc, n_cores):
    import jax
    import jax.numpy as jnp
    from jax.sharding import Mesh, NamedSharding, PartitionSpec
    from jax.experimental.shard_map import shard_map
    import concourse.mybir as mybir
    from concourse import bass2jax
    from concourse.bass2jax import _bass_exec_p, partition_id_tensor

    bass2jax.install_neuronx_cc_hook()
    partition_name = (nc.partition_id_tensor.name
                      if nc.partition_id_tensor else None)

    in_names, out_names, out_avals = [], [], []
    for alloc in nc.m.functions[0].allocations:
        if not isinstance(alloc, mybir.MemoryLocationSet):
            continue
        name = alloc.memorylocations[0].name
        if alloc.kind == "ExternalInput":
            if name != partition_name:
                in_names.append(name)
        elif alloc.kind == "ExternalOutput":
            shape = tuple(alloc.tensor_shape)
            dtype = mybir.dt.np(alloc.dtype)
            out_names.append(name)
            out_avals.append(jax.core.ShapedArray(shape, dtype))
    in_names_all = list(in_names) + out_names
    if partition_name is not None:
        in_names_all = in_names_all + [partition_name]

    # in_names order follows allocation order: x_in, annT_b, amT_b
    assert in_names == ["x_in", "annT_b", "amT_b"], in_names
    assert out_names == ["cout"], out_names

    devices = jax.devices()[:n_cores]
    mesh = Mesh(np.asarray(devices), ("core",))
    shard_rows = NamedSharding(mesh, PartitionSpec("core"))
    shard_rep = NamedSharding(mesh, PartitionSpec())

    def _body(x, ann, am, *zeros):
        operands = [x, ann, am, *zeros]
        if partition_name is not None:
            operands.append(partition_id_tensor())
        outs = _bass_exec_p.bind(
            *operands,
            out_avals=tuple(out_avals),
            in_names=tuple(in_names_all),
            out_names=tuple(out_names),
            lowering_input_output_aliases=(),
            sim_require_finite=True,
            sim_require_nnan=True,
            nc=nc,
        )
        return outs[0]

    n_outs = len(out_avals)
    sharded = jax.jit(
        shard_map(_body, mesh=mesh,
                  in_specs=(PartitionSpec("core"), PartitionSpec(),
                            PartitionSpec())
                  + (PartitionSpec("core"),) * n_outs,
                  out_specs=PartitionSpec("core"),
                  check_rep=False))

    def _zeros():
        # cached, undonated: the kernel fully writes cout every call, so
        # the initial content of the bound output buffers never matters.
        if "zeros" not in _state:
            z = tuple(
                jax.device_put(
                    np.zeros((n_cores * a.shape[0], *a.shape[1:]), a.dtype),
                    shard_rows)
                for a in out_avals)
            jax.block_until_ready(z)
            _state["zeros"] = z
        return _state["zeros"]

    def run(x_np, annd, amd):
        xd = jax.device_put(x_np, shard_rows)
        return sharded(xd, annd, amd, *_zeros())

    run.mesh = mesh
    run.shard_rep = shard_rep
    return run


# --------------------------------------------------------------------------
# host-side tables (bf16, int-mod angle trick; device-cached)
# --------------------------------------------------------------------------
def _tables_np(expkM, expkN):
    import ml_dtypes
    n = _SZ
    i = np.arange(n)
    pm = np.where(i < (n + 1) // 2, 2 * i, 2 * (n - i) - 1)
    pinv = np.empty(n, dtype=np.int64)
    pinv[pm] = i
    # annT[j, v] = 2*(cos(th)*eNr[v] + sin(th)*eNi[v]), th = 2pi*pinv[j]*v/n
    # exact angle via integer mod: th = 2pi*((pinv[j]*v) mod n)/n
    idx = (pinv[:, None].astype(np.int64) * i[None, :]) % n  # (n, n)
    lut_c = np.cos((2.0 * np.pi / n) * np.arange(n))
    lut_s = np.sin((2.0 * np.pi / n) * np.arange(n))
    Cp = lut_c[idx]
    Sp = lut_s[idx]
    eNr = expkN[:, 0].astype(np.float64)
    eNi = expkN[:, 1].astype(np.float64)
    eMr = expkM[:, 0].astype(np.float64)
    eMi = expkM[:, 1].astype(np.float64)
    annT = (2.0 * (Cp * eNr[None, :] + Sp * eNi[None, :]))
    amT = (0.5 * (Cp * eMr[None, :] + Sp * eMi[None, :]))
    return (annT.astype(ml_dtypes.bfloat16), amT.astype(ml_dtypes.bfloat16))


def _tables(expkM, expkN, run):
    import jax
    key = (expkM.tobytes(), expkN.tobytes())
    cached = _state.get("tables")
    if cached is not None and cached[0] == key:
        return cached[1], cached[2]
    annT_b, amT_b = _tables_np(expkM, expkN)
    annd = jax.device_put(annT_b, run.shard_rep)
    amd = jax.device_put(amT_b, run.shard_rep)
    jax.block_until_ready((annd, amd))
    _state["tables"] = (key, annd, amd)
    return annd, amd


def kernel(x, expkM, expkN, M, N):
    x = np.asarray(x, dtype=np.float32)
    expkM = np.asarray(expkM, dtype=np.float32)
    expkN = np.asarray(expkN, dtype=np.float32)
    assert x.shape == (_SZ, _SZ)

    if "run" not in _state:
        nc = _build_bass()
        _state["nc"] = nc
        _state["run"] = _build_runner(nc, _NCORES)
    run = _state["run"]
    annd, amd = _tables(expkM, expkN, run)
    out = run(x, annd, amd)
    return np.asarray(out)


# revision 9
# speedup vs baseline: 1402.7287x; 1.3865x over previous
"""2D DCT-II (4096x4096) on 8 Trainium2 NeuronCores (axon/PJRT SPMD).

Math: C = A_M @ x @ A_N^T with A tables folding the Makhoul permutation and
expk twiddles (annT[j, v] = A_N[v, j], amT[g, u] = A_M[u, g]).

Fold trick (halves both matmuls): rows of annT pair up as
  annT[pm[i+n/2], v] = (-1)^v * annT[pm[i], v],   pm[i] = 2i, pm[i+n/2] = 4095-2i
so with e_i = x[:, 2i] + x[:, 4095-2i], d_i = x[:, 2i] - x[:, 4095-2i]:
  W[:, even v] = e @ annT_f[:, even v],  W[:, odd v] = d @ annT_f[:, odd v]
where annT_f = annT[0::2] is the 2048-row folded table. Same structure for
amT in the second (row-transform) phase. Folds are computed on TensorE with
six constant 128x128 comb matrices (contraction pairs live on partitions).

Device schedule (core k of 8, all matmuls bf16 into fp32 PSUM):
  phase 0: DMA x[rows_k], TensorE-transpose to xw[c-part, (ct, r)],
           fold into xe/xd [i-part, (t, r)] (i < 2048).
  phase 1: W_k rows: even-v from xe @ ann_fe, odd-v from xd @ ann_fo;
           blocks stored v-permuted: w_send[j, r, 0:256]=evens, [256:512]=odds.
  AllToAll #1 (bf16); phase 2: fold W rows g (same combs) -> we/wd;
           C[even u, cols_k] = am_fe.T @ we, odd u from am_fo.T @ wd,
           written to c_send[j] with stride-2 rows (true row order).
  AllToAll #2 (bf16); final: convert bf16->fp32 and un-permute v within
           each 512-block: cout[r, 512m+2c(+1)] = c_recv[m, r, (par*256)+c].
Host: no transposes/concats; x device_put row-sharded; output of shard_map
IS the full (4096,4096) C. Tables device-cached (32MB bf16 total).
"""
import numpy as np

_NCORES = 8
_SZ = 4096
_RPC = _SZ // _NCORES  # 512 rows/cols per core
_KT = _SZ // 128       # 32 c-tiles (unfolded)
_KT2 = _KT // 2        # 16 folded contraction tiles

_state = {}


def _fold_mats():
    """Six 128x128 bf16 comb matrices for the fold matmuls.
    e-tile t chain: M0@xw[2t] + M1@xw[2t+1] + M2@xw[31-2t] + M3@xw[30-2t]
    d-tile t chain: M0, M1, -M2, -M3."""
    import ml_dtypes
    M = np.zeros((6, 128, 128), np.float32)
    for i in range(64):
        M[0, 2 * i, i] = 1.0            # g' = 2i', i' < 64
        M[1, 2 * i, 64 + i] = 1.0       # g' = 2(i'-64), i' >= 64
    for i in range(64):
        M[2, 127 - 2 * i, i] = 1.0      # mirror, i' < 64
        M[3, 255 - 2 * (64 + i) - 0, 64 + i] = 1.0  # g' = 255-2i', i' >= 64
    M[4] = -M[2]
    M[5] = -M[3]
    return M.astype(ml_dtypes.bfloat16)


# --------------------------------------------------------------------------
# Bass kernel
# --------------------------------------------------------------------------
def _build_bass():
    import concourse.bacc as bacc
    import concourse.mybir as mybir
    from concourse.tile import TileContext
    from concourse import masks

    fp32 = mybir.dt.float32
    bf16 = mybir.dt.bfloat16
    nc = bacc.Bacc("TRN2", target_bir_lowering=False, debug=False,
                   num_devices=_NCORES)
    x_in = nc.declare_dram_parameter("x_in", [_RPC, _SZ], fp32,
                                     isOutput=False)
    ann_fe = nc.declare_dram_parameter("ann_fe", [_SZ // 2, _SZ // 2], bf16,
                                       isOutput=False)
    ann_fo = nc.declare_dram_parameter("ann_fo", [_SZ // 2, _SZ // 2], bf16,
                                       isOutput=False)
    am_fe = nc.declare_dram_parameter("am_fe", [_SZ // 2, _SZ // 2], bf16,
                                      isOutput=False)
    am_fo = nc.declare_dram_parameter("am_fo", [_SZ // 2, _SZ // 2], bf16,
                                      isOutput=False)
    fold_m = nc.declare_dram_parameter("fold_m", [6, 128, 128], bf16,
                                       isOutput=False)
    cout = nc.declare_dram_parameter("cout", [_RPC, _SZ], fp32,
                                     isOutput=True)

    w_send = nc.dram_tensor("w_send", [_NCORES, _RPC, _RPC], bf16)
    w_recv = nc.dram_tensor("w_recv", [_NCORES, _RPC, _RPC], bf16)
    c_send = nc.dram_tensor("c_send", [_NCORES, _RPC, _RPC], bf16)
    c_recv = nc.dram_tensor("c_recv", [_NCORES, _RPC, _RPC], bf16)

    RT = _RPC // 128  # 4 row tiles per core

    def fold_chain(ps, fm, neg, src, t, width):
        """Accumulate the 4-matmul fold chain for tile t into psum ps.
        fm: SBUF tile [128, 6*128] holding combs column-blocked."""
        if neg:  # d-chain: negated mirror combs
            pairs = [(0, 2 * t), (1, 2 * t + 1), (4, 31 - 2 * t),
                     (5, 30 - 2 * t)]
        else:
            pairs = [(0, 2 * t), (1, 2 * t + 1), (2, 31 - 2 * t),
                     (3, 30 - 2 * t)]
        for n_i, (mi, gt) in enumerate(pairs):
            nc.tensor.matmul(
                ps[:], fm[:, mi * 128:(mi + 1) * 128],
                src[:, gt * width:(gt + 1) * width],
                start=(n_i == 0), stop=(n_i == 3))

    with TileContext(nc) as tc:
        with tc.tile_pool(name="fm", bufs=1) as fm_pool:
            fmt = fm_pool.tile([128, 6 * 128], bf16)
            nc.sync.dma_start(
                out=fmt[:].rearrange("p (m c) -> p m c", m=6),
                in_=fold_m[:].rearrange("m p c -> p m c"))
            fm = fmt[:]

            # ============ phase 0: transpose + fold x ============
            with (
                tc.tile_pool(name="ident", bufs=1) as ident_pool,
                tc.tile_pool(name="xev", bufs=1) as xe_pool,
                tc.tile_pool(name="xw", bufs=1) as xw_pool,
            ):
                identf = ident_pool.tile([128, 128], fp32)
                masks.make_identity(nc, identf[:])
                # xe: [i-part, (t, r)] folded sums; xd: diffs  (16 KB/part ea)
                xe = xe_pool.tile([128, _KT2 * _RPC], bf16)
                xd = xe_pool.tile([128, _KT2 * _RPC], bf16)
                xw = xw_pool.tile([128, _KT * _RPC], bf16)  # 32 KB/part

                with (
                    tc.tile_pool(name="xf", bufs=1) as xf_pool,
                    tc.tile_pool(name="pst", bufs=3, space="PSUM") as pst_pool,
                    tc.tile_pool(name="psf", bufs=3, space="PSUM") as psf_pool,
                ):
                    xf = xf_pool.tile([128, RT * _SZ], fp32)  # 64 KB/part
                    for rt in range(RT):
                        nc.sync.dma_start(
                            out=xf[:, rt * _SZ:(rt + 1) * _SZ],
                            in_=x_in[rt * 128:(rt + 1) * 128, :])
                    for ct in range(_KT):
                        ps = pst_pool.tile([128, _RPC], fp32, tag="pst")
                        for rt in range(RT):
                            nc.tensor.transpose(
                                ps[:, rt * 128:(rt + 1) * 128],
                                xf[:, rt * _SZ + ct * 128:
                                      rt * _SZ + ct * 128 + 128],
                                identf[:])
                        nc.vector.tensor_copy(
                            xw[:, ct * _RPC:(ct + 1) * _RPC], ps[:])
                    for t in range(_KT2):
                        pe = psf_pool.tile([128, _RPC], fp32, tag="psf")
                        fold_chain(pe, fm, False, xw, t, _RPC)
                        nc.vector.tensor_copy(
                            xe[:, t * _RPC:(t + 1) * _RPC], pe[:])
                        pd = psf_pool.tile([128, _RPC], fp32, tag="psf")
                        fold_chain(pd, fm, True, xw, t, _RPC)
                        nc.vector.tensor_copy(
                            xd[:, t * _RPC:(t + 1) * _RPC], pd[:])

                # ============ phase 1: W = x @ A_N^T (folded) ============
                with (
                    tc.tile_pool(name="an", bufs=3) as an_pool,
                    tc.tile_pool(name="ps1", bufs=8, space="PSUM") as ps1_pool,
                    tc.tile_pool(name="ev1", bufs=4) as ev1_pool,
                ):
                    for br, (tab, src) in enumerate(
                            [(ann_fe, xe), (ann_fo, xd)]):
                        for vp in range(4):  # 512 q's per panel
                            an = an_pool.tile([128, _KT2 * _RPC], bf16,
                                              tag="an")
                            nc.sync.dma_start(
                                out=an[:].rearrange("p (t q) -> p t q",
                                                    t=_KT2),
                                in_=tab[:, vp * _RPC:(vp + 1) * _RPC]
                                .rearrange("(t p) q -> p t q", p=128))
                            for rt in range(RT):
                                ps = ps1_pool.tile([128, _RPC], fp32,
                                                   tag="ps")
                                for k in range(_KT2):
                                    nc.tensor.matmul(
                                        ps[:],
                                        src[:, k * _RPC + rt * 128:
                                               k * _RPC + rt * 128 + 128],
                                        an[:, k * _RPC:(k + 1) * _RPC],
                                        start=(k == 0), stop=(k == _KT2 - 1))
                                ev = ev1_pool.tile([128, _RPC], bf16,
                                                   tag="ev")
                                nc.vector.tensor_copy(ev[:], ps[:])
                                c0 = br * 256  # evens 0:256, odds 256:512
                                nc.sync.dma_start(
                                    out=w_send[2 * vp,
                                               rt * 128:(rt + 1) * 128,
                                               c0:c0 + 256],
                                    in_=ev[:, 0:256])
                                nc.sync.dma_start(
                                    out=w_send[2 * vp + 1,
                                               rt * 128:(rt + 1) * 128,
                                               c0:c0 + 256],
                                    in_=ev[:, 256:512])

            # ============ exchange 1 ============
            nc.gpsimd.collective_compute(
                "AllToAll",
                mybir.AluOpType.bypass,
                ins=[w_send[:]],
                outs=[w_recv[:]],
                replica_groups=[list(range(_NCORES))],
            )

            # ============ phase 2: C = A_M @ W (folded) ============
            with (
                tc.tile_pool(name="wev", bufs=1) as we_pool,
            ):
                # we: [i-part, (t, v)] folded sums of W rows; wd: diffs
                we = we_pool.tile([128, _KT2 * _RPC], bf16)
                wd = we_pool.tile([128, _KT2 * _RPC], bf16)
                with (
                    tc.tile_pool(name="wr", bufs=1) as wr_pool,
                    tc.tile_pool(name="psf2", bufs=4, space="PSUM")
                    as psf2_pool,
                ):
                    wr = wr_pool.tile([128, _KT * _RPC], bf16)  # 32 KB/part
                    nc.sync.dma_start(
                        out=wr[:].rearrange("p (m s v) -> p m s v",
                                            m=_NCORES, s=4),
                        in_=w_recv[:].rearrange("m (s p) v -> p m s v",
                                                p=128))
                    for t in range(_KT2):
                        pe = psf2_pool.tile([128, _RPC], fp32, tag="psf2")
                        fold_chain(pe, fm, False, wr, t, _RPC)
                        nc.vector.tensor_copy(
                            we[:, t * _RPC:(t + 1) * _RPC], pe[:])
                        pd = psf2_pool.tile([128, _RPC], fp32, tag="psf2")
                        fold_chain(pd, fm, True, wr, t, _RPC)
                        nc.vector.tensor_copy(
                            wd[:, t * _RPC:(t + 1) * _RPC], pd[:])

                with (
                    tc.tile_pool(name="am", bufs=3) as am_pool,
                    tc.tile_pool(name="ps2", bufs=8, space="PSUM") as ps2_pool,
                    tc.tile_pool(name="ev2", bufs=4) as ev2_pool,
                ):
                    for br, (tab, src) in enumerate([(am_fe, we),
                                                     (am_fo, wd)]):
                        for up in range(4):  # 512 half-u's per panel
                            am = am_pool.tile([128, _KT2 * _RPC], bf16,
                                              tag="am")
                            nc.sync.dma_start(
                                out=am[:].rearrange("p (t q) -> p t q",
                                                    t=_KT2),
                                in_=tab[:, up * _RPC:(up + 1) * _RPC]
                                .rearrange("(t p) q -> p t q", p=128))
                            for sub in range(RT):
                                e = up * RT + sub  # u-tile index 0..15
                                ps = ps2_pool.tile([128, _RPC], fp32,
                                                   tag="ps2")
                                for k in range(_KT2):
                                    nc.tensor.matmul(
                                        ps[:],
                                        am[:, k * _RPC + sub * 128:
                                               k * _RPC + sub * 128 + 128],
                                        src[:, k * _RPC:(k + 1) * _RPC],
                                        start=(k == 0), stop=(k == _KT2 - 1))
                                ev = ev2_pool.tile([128, _RPC], bf16,
                                                   tag="ev2")
                                nc.vector.tensor_copy(ev[:], ps[:])
                                # u = 2*(128e + i) + br ; block j = e//2,
                                # rows = 256*(e%2) + 2*i + br
                                csv = c_send[:].rearrange(
                                    "j (r2 par) c -> j r2 par c", par=2)
                                nc.sync.dma_start(
                                    out=csv[e // 2,
                                            (e % 2) * 128:(e % 2) * 128 + 128,
                                            br, :],
                                    in_=ev[:])

            # ============ exchange 2 ============
            nc.gpsimd.collective_compute(
                "AllToAll",
                mybir.AluOpType.bypass,
                ins=[c_send[:]],
                outs=[c_recv[:]],
                replica_groups=[list(range(_NCORES))],
            )

            # ============ final: un-permute v, convert to fp32 ============
            with (
                tc.tile_pool(name="cr", bufs=2) as cr_pool,
                tc.tile_pool(name="co", bufs=2) as co_pool,
            ):
                for rt in range(RT):
                    cr = cr_pool.tile([128, _NCORES * _RPC], bf16, tag="cr")
                    nc.sync.dma_start(
                        out=cr[:].rearrange("p (m pc) -> p m pc", m=_NCORES),
                        in_=c_recv[:, rt * 128:(rt + 1) * 128, :]
                        .rearrange("m p pc -> p m pc"))
                    co = co_pool.tile([128, _SZ], fp32, tag="co")
                    # cout[p, 512m + 2c + par] = cr[p, m, par*256 + c]
                    nc.vector.tensor_copy(
                        co[:].rearrange("p (m c par) -> p m par c",
                                        par=2, c=256),
                        cr[:].rearrange("p (m par c) -> p m par c",
                                        par=2, c=256))
                    nc.sync.dma_start(
                        out=cout[rt * 128:(rt + 1) * 128, :], in_=co[:])

    nc.compile()
    return nc


# --------------------------------------------------------------------------
# PJRT SPMD runner (compile once, run many; single jit per call)
# --------------------------------------------------------------------------
def _build_runner(nc, n_cores):
    import jax
    from jax.sharding import Mesh, NamedSharding, PartitionSpec
    from jax.experimental.shard_map import shard_map
    import concourse.mybir as mybir
    from concourse import bass2jax
    from concourse.bass2jax import _bass_exec_p, partition_id_tensor

    bass2jax.install_neuronx_cc_hook()
    partition_name = (nc.partition_id_tensor.name
                      if nc.partition_id_tensor else None)

    in_names, out_names, out_avals = [], [], []
    for alloc in nc.m.functions[0].allocations:
        if not isinstance(alloc, mybir.MemoryLocationSet):
            continue
        name = alloc.memorylocations[0].name
        if alloc.kind == "ExternalInput":
            if name != partition_name:
                in_names.append(name)
        elif alloc.kind == "ExternalOutput":
            shape = tuple(alloc.tensor_shape)
            dtype = mybir.dt.np(alloc.dtype)
            out_names.append(name)
            out_avals.append(jax.core.ShapedArray(shape, dtype))
    in_names_all = list(in_names) + out_names
    if partition_name is not None:
        in_names_all = in_names_all + [partition_name]

    assert in_names[0] == "x_in", in_names
    assert out_names == ["cout"], out_names
    n_rep = len(in_names) - 1  # replicated table inputs

    devices = jax.devices()[:n_cores]
    mesh = Mesh(np.asarray(devices), ("core",))
    shard_rows = NamedSharding(mesh, PartitionSpec("core"))
    shard_rep = NamedSharding(mesh, PartitionSpec())

    def _body(*args):
        operands = list(args)
        if partition_name is not None:
            operands.append(partition_id_tensor())
        outs = _bass_exec_p.bind(
            *operands,
            out_avals=tuple(out_avals),
            in_names=tuple(in_names_all),
            out_names=tuple(out_names),
            lowering_input_output_aliases=(),
            sim_require_finite=True,
            sim_require_nnan=True,
            nc=nc,
        )
        return outs[0]

    n_outs = len(out_avals)
    sharded = jax.jit(
        shard_map(_body, mesh=mesh,
                  in_specs=(PartitionSpec("core"),)
                  + (PartitionSpec(),) * n_rep
                  + (PartitionSpec("core"),) * n_outs,
                  out_specs=PartitionSpec("core"),
                  check_rep=False))

    def _zeros():
        # cached, undonated: the kernel fully writes cout every call, so
        # the initial content of the bound output buffers never matters.
        if "zeros" not in _state:
            z = tuple(
                jax.device_put(
                    np.zeros((n_cores * a.shape[0], *a.shape[1:]), a.dtype),
                    shard_rows)
                for a in out_avals)
            jax.block_until_ready(z)
            _state["zeros"] = z
        return _state["zeros"]

    def run(x_np, tables):
        xd = jax.device_put(x_np, shard_rows)
        return sharded(xd, *tables, *_zeros())

    run.in_names = in_names
    run.shard_rep = shard_rep
    return run


# --------------------------------------------------------------------------
# host-side tables (bf16 folded; device-cached)
# --------------------------------------------------------------------------
def _tables_np(expkM, expkN):
    import ml_dtypes
    n = _SZ
    h = n // 2
    i = np.arange(h)
    v = np.arange(n)
    # pinv[2i] = i  ->  folded row i corresponds to original column j = 2i
    idx = (i[:, None] * v[None, :]) % n  # (h, n)
    lut_c = np.cos((2.0 * np.pi / n) * np.arange(n))
    lut_s = np.sin((2.0 * np.pi / n) * np.arange(n))
    Cp = lut_c[idx]
    Sp = lut_s[idx]
    eNr = expkN[:, 0].astype(np.float64)
    eNi = expkN[:, 1].astype(np.float64)
    eMr = expkM[:, 0].astype(np.float64)
    eMi = expkM[:, 1].astype(np.float64)
    annT_f = 2.0 * (Cp * eNr[None, :] + Sp * eNi[None, :])
    amT_f = 0.5 * (Cp * eMr[None, :] + Sp * eMi[None, :])
    bf = ml_dtypes.bfloat16
    return (np.ascontiguousarray(annT_f[:, 0::2]).astype(bf),
            np.ascontiguousarray(annT_f[:, 1::2]).astype(bf),
            np.ascontiguousarray(amT_f[:, 0::2]).astype(bf),
            np.ascontiguousarray(amT_f[:, 1::2]).astype(bf),
            _fold_mats())


def _tables(expkM, expkN, run):
    import jax
    key = (expkM.tobytes(), expkN.tobytes())
    cached = _state.get("tables")
    if cached is not None and cached[0] == key:
        return cached[1]
    tabs = _tables_np(expkM, expkN)
    named = dict(zip(["ann_fe", "ann_fo", "am_fe", "am_fo", "fold_m"], tabs))
    devs = tuple(jax.device_put(named[nm], run.shard_rep)
                 for nm in run.in_names[1:])
    jax.block_until_ready(devs)
    _state["tables"] = (key, devs)
    return devs


def kernel(x, expkM, expkN, M, N):
    x = np.asarray(x, dtype=np.float32)
    expkM = np.asarray(expkM, dtype=np.float32)
    expkN = np.asarray(expkN, dtype=np.float32)
    assert x.shape == (_SZ, _SZ)

    if "run" not in _state:
        nc = _build_bass()
        _state["nc"] = nc
        _state["run"] = _build_runner(nc, _NCORES)
    run = _state["run"]
    tabs = _tables(expkM, expkN, run)
    out = run(x, tabs)
    return np.asarray(out)


# revision 13
# speedup vs baseline: 1883.1033x; 1.3425x over previous
"""2D DCT-II (4096x4096) on 8 Trainium2 NeuronCores (axon/PJRT SPMD).

Math: C = A_M @ x @ A_N^T with A tables folding the Makhoul permutation and
expk twiddles (annT[j, v] = A_N[v, j], amT[g, u] = A_M[u, g]).

Fold trick (halves both matmuls): rows of annT pair up as
  annT[pm[i+n/2], v] = (-1)^v * annT[pm[i], v],   pm[i] = 2i, pm[i+n/2] = 4095-2i
so with e_i = x[:, 2i] + x[:, 4095-2i], d_i = x[:, 2i] - x[:, 4095-2i]:
  W[:, even v] = e @ annT_f[:, even v],  W[:, odd v] = d @ annT_f[:, odd v]
where annT_f = annT[0::2] is the 2048-row folded table. Same structure for
amT in the second (row-transform) phase. Folds are computed on TensorE with
six constant 128x128 comb matrices (contraction pairs live on partitions).

Device schedule (core k of 8, all matmuls bf16 into fp32 PSUM):
  phase 0: DMA x[rows_k], TensorE-transpose to xw[c-part, (ct, r)],
           fold into xe/xd [i-part, (t, r)] (i < 2048).
  phase 1: W_k rows: even-v from xe @ ann_fe, odd-v from xd @ ann_fo;
           blocks stored v-permuted: w_send[j, r, 0:256]=evens, [256:512]=odds.
  AllToAll #1 (bf16); phase 2: fold W rows g (same combs) -> we/wd;
           C[even u, cols_k] = am_fe.T @ we, odd u from am_fo.T @ wd,
           written to c_send[j] with stride-2 rows (true row order).
  AllToAll #2 (bf16); final: convert bf16->fp32 and un-permute v within
           each 512-block: cout[r, 512m+2c(+1)] = c_recv[m, r, (par*256)+c].
Host: no transposes/concats; x device_put row-sharded; output of shard_map
IS the full (4096,4096) C. Tables device-cached (32MB bf16 total).
"""
import numpy as np

_NCORES = 8
_SZ = 4096
_RPC = _SZ // _NCORES  # 512 rows/cols per core
_KT = _SZ // 128       # 32 c-tiles (unfolded)
_KT2 = _KT // 2        # 16 folded contraction tiles

_state = {}


def _fold_mats():
    """Six 128x128 bf16 comb matrices for the fold matmuls.
    e-tile t chain: M0@xw[2t] + M1@xw[2t+1] + M2@xw[31-2t] + M3@xw[30-2t]
    d-tile t chain: M0, M1, -M2, -M3."""
    import ml_dtypes
    M = np.zeros((6, 128, 128), np.float32)
    for i in range(64):
        M[0, 2 * i, i] = 1.0            # g' = 2i', i' < 64
        M[1, 2 * i, 64 + i] = 1.0       # g' = 2(i'-64), i' >= 64
    for i in range(64):
        M[2, 127 - 2 * i, i] = 1.0      # mirror, i' < 64
        M[3, 255 - 2 * (64 + i) - 0, 64 + i] = 1.0  # g' = 255-2i', i' >= 64
    M[4] = -M[2]
    M[5] = -M[3]
    return M.astype(ml_dtypes.bfloat16)


# --------------------------------------------------------------------------
# Bass kernel
# --------------------------------------------------------------------------
def _build_bass():
    import concourse.bacc as bacc
    import concourse.mybir as mybir
    from concourse.tile import TileContext
    from concourse import masks

    fp32 = mybir.dt.float32
    bf16 = mybir.dt.bfloat16
    nc = bacc.Bacc("TRN2", target_bir_lowering=False, debug=False,
                   num_devices=_NCORES)
    x_in = nc.declare_dram_parameter("x_in", [_RPC, _SZ], fp32,
                                     isOutput=False)
    ann_fe = nc.declare_dram_parameter("ann_fe", [_SZ // 2, _SZ // 2], bf16,
                                       isOutput=False)
    ann_fo = nc.declare_dram_parameter("ann_fo", [_SZ // 2, _SZ // 2], bf16,
                                       isOutput=False)
    am_fe = nc.declare_dram_parameter("am_fe", [_SZ // 2, _SZ // 2], bf16,
                                      isOutput=False)
    am_fo = nc.declare_dram_parameter("am_fo", [_SZ // 2, _SZ // 2], bf16,
                                      isOutput=False)
    fold_m = nc.declare_dram_parameter("fold_m", [6, 128, 128], bf16,
                                       isOutput=False)
    cout = nc.declare_dram_parameter("cout", [_RPC, _SZ], fp32,
                                     isOutput=True)

    w_send = [nc.dram_tensor(f"w_send{h}", [_NCORES, _RPC, 256], bf16)
              for h in range(2)]
    w_recv = [nc.dram_tensor(f"w_recv{h}", [_NCORES, _RPC, 256], bf16)
              for h in range(2)]
    c_send = [nc.dram_tensor(f"c_send{h}", [_NCORES, _RPC, 256], bf16)
              for h in range(2)]
    c_recv = [nc.dram_tensor(f"c_recv{h}", [_NCORES, _RPC, 256], bf16)
              for h in range(2)]

    RT = _RPC // 128  # 4 row tiles per core

    def fold_chain(ps, fm, neg, src, t, width):
        """Accumulate the 4-matmul fold chain for tile t into psum ps.
        fm: SBUF tile [128, 6*128] holding combs column-blocked."""
        if neg:  # d-chain: negated mirror combs
            pairs = [(0, 2 * t), (1, 2 * t + 1), (4, 31 - 2 * t),
                     (5, 30 - 2 * t)]
        else:
            pairs = [(0, 2 * t), (1, 2 * t + 1), (2, 31 - 2 * t),
                     (3, 30 - 2 * t)]
        for n_i, (mi, gt) in enumerate(pairs):
            nc.tensor.matmul(
                ps[:], fm[:, mi * 128:(mi + 1) * 128],
                src[:, gt * width:(gt + 1) * width],
                start=(n_i == 0), stop=(n_i == 3))

    with TileContext(nc) as tc:
        with (
            tc.tile_pool(name="fm", bufs=1) as fm_pool,
            tc.tile_pool(name="pre", bufs=1) as pre_pool,
        ):
            fmt = fm_pool.tile([128, 6 * 128], bf16)
            nc.sync.dma_start(
                out=fmt[:].rearrange("p (m c) -> p m c", m=6),
                in_=fold_m[:].rearrange("m p c -> p m c"))
            fm = fmt[:]
            # phase-2 half-a working set, preallocated so its loads/folds
            # overlap phase 1 (no pool-swap serialization):
            wrt = [pre_pool.tile([128, _KT * 256], bf16, name=f"wrt{h}")
                   for h in range(2)]
            wet = pre_pool.tile([128, _KT2 * 256], bf16)
            wdt = pre_pool.tile([128, _KT2 * 256], bf16)
            we2t = pre_pool.tile([128, 8 * 256], bf16)
            wd2t = pre_pool.tile([128, 8 * 256], bf16)
            amee = [pre_pool.tile([128, 8 * _RPC], bf16, name=f"amee{u}")
                    for u in range(2)]
            ameo = [pre_pool.tile([128, 8 * _RPC], bf16, name=f"ameo{u}")
                    for u in range(2)]

            # ============ phase 0: transpose + fold x ============
            with (
                tc.tile_pool(name="ident", bufs=1) as ident_pool,
                tc.tile_pool(name="xev", bufs=1) as xe_pool,
                tc.tile_pool(name="xw", bufs=1) as xw_pool,
            ):
                identf = ident_pool.tile([128, 128], fp32)
                masks.make_identity(nc, identf[:])
                # xe: [i-part, (t, r)] folded sums; xd: diffs  (16 KB/part ea)
                xe = xe_pool.tile([128, _KT2 * _RPC], bf16)
                xd = xe_pool.tile([128, _KT2 * _RPC], bf16)
                xw = xw_pool.tile([128, _KT * _RPC], bf16)  # 32 KB/part

                with (
                    tc.tile_pool(name="xf", bufs=2) as xf_pool,
                    tc.tile_pool(name="pst", bufs=3, space="PSUM") as pst_pool,
                    tc.tile_pool(name="psf", bufs=3, space="PSUM") as psf_pool,
                ):
                    for rt in range(RT):
                        nc.sync.dma_start(
                            out=xf[:, rt * _SZ:(rt + 1) * _SZ],
                            in_=x_in[rt * 128:(rt + 1) * 128, :])
                    for ct in range(_KT):
                        ps = pst_pool.tile([128, _RPC], fp32, tag="pst")
                        for rt in range(RT):
                            nc.tensor.transpose(
                                ps[:, rt * 128:(rt + 1) * 128],
                                xf[:, rt * _SZ + ct * 128:
                                      rt * _SZ + ct * 128 + 128],
                                identf[:])
                        nc.vector.tensor_copy(
                            xw[:, ct * _RPC:(ct + 1) * _RPC], ps[:])
                    for t in range(_KT2):
                        pe = psf_pool.tile([128, _RPC], fp32, tag="psf")
                        fold_chain(pe, fm, False, xw, t, _RPC)
                        nc.vector.tensor_copy(
                            xe[:, t * _RPC:(t + 1) * _RPC], pe[:])
                        pd = psf_pool.tile([128, _RPC], fp32, tag="psf")
                        fold_chain(pd, fm, True, xw, t, _RPC)
                        nc.vector.tensor_copy(
                            xd[:, t * _RPC:(t + 1) * _RPC], pd[:])

                # ============ phase 1: W = x @ A_N^T (folded) ============
                with (
                    tc.tile_pool(name="an", bufs=3) as an_pool,
                    tc.tile_pool(name="ps1", bufs=6, space="PSUM") as ps1_pool,
                    tc.tile_pool(name="psp", bufs=2, space="PSUM") as psp_pool,
                    tc.tile_pool(name="ev1", bufs=4) as ev1_pool,
                ):
                    for br, (tab, src) in enumerate(
                            [(ann_fe, xe), (ann_fo, xd)]):
                        for vp in range(4):  # 512 q's per panel
                            an = an_pool.tile([128, _KT2 * _RPC], bf16,
                                              tag="an")
                            nc.sync.dma_start(
                                out=an[:].rearrange("p (t q) -> p t q",
                                                    t=_KT2),
                                in_=tab[:, vp * _RPC:(vp + 1) * _RPC]
                                .rearrange("(t p) q -> p t q", p=128))
                            for rt in range(RT):
                                ps = ps1_pool.tile([128, _RPC], fp32,
                                                   tag="ps")
                                for k in range(_KT2):
                                    nc.tensor.matmul(
                                        ps[:],
                                        src[:, k * _RPC + rt * 128:
                                               k * _RPC + rt * 128 + 128],
                                        an[:, k * _RPC:(k + 1) * _RPC],
                                        start=(k == 0), stop=(k == _KT2 - 1))
                                ev = ev1_pool.tile([128, _RPC], bf16,
                                                   tag="ev")
                                nc.vector.tensor_copy(ev[:], ps[:])
                                c0 = br * 256  # evens 0:256, odds 256:512
                                nc.sync.dma_start(
                                    out=w_send[2 * vp,
                                               rt * 128:(rt + 1) * 128,
                                               c0:c0 + 256],
                                    in_=ev[:, 0:256])
                                nc.sync.dma_start(
                                    out=w_send[2 * vp + 1,
                                               rt * 128:(rt + 1) * 128,
                                               c0:c0 + 256],
                                    in_=ev[:, 256:512])

            # ============ exchange 1 ============
            nc.gpsimd.collective_compute(
                "AllToAll",
                mybir.AluOpType.bypass,
                ins=[w_send[:]],
                outs=[w_recv[:]],
                replica_groups=[list(range(_NCORES))],
            )

            # ============ phase 2: C = A_M @ W (folded) ============
            with (
                tc.tile_pool(name="wev", bufs=1) as we_pool,
            ):
                # we: [i-part, (t, v)] folded sums of W rows; wd: diffs
                we = we_pool.tile([128, _KT2 * _RPC], bf16)
                wd = we_pool.tile([128, _KT2 * _RPC], bf16)
                with (
                    tc.tile_pool(name="wr", bufs=1) as wr_pool,
                    tc.tile_pool(name="psf2", bufs=4, space="PSUM")
                    as psf2_pool,
                ):
                    wr = wr_pool.tile([128, _KT * _RPC], bf16)  # 32 KB/part
                    nc.sync.dma_start(
                        out=wr[:].rearrange("p (m s v) -> p m s v",
                                            m=_NCORES, s=4),
                        in_=w_recv[:].rearrange("m (s p) v -> p m s v",
                                                p=128))
                    for t in range(_KT2):
                        pe = psf2_pool.tile([128, _RPC], fp32, tag="psf2")
                        fold_chain(pe, fm, False, wr, t, _RPC)
                        nc.vector.tensor_copy(
                            we[:, t * _RPC:(t + 1) * _RPC], pe[:])
                        pd = psf2_pool.tile([128, _RPC], fp32, tag="psf2")
                        fold_chain(pd, fm, True, wr, t, _RPC)
                        nc.vector.tensor_copy(
                            wd[:, t * _RPC:(t + 1) * _RPC], pd[:])

                with (
                    tc.tile_pool(name="am", bufs=3) as am_pool,
                    tc.tile_pool(name="ps2", bufs=8, space="PSUM") as ps2_pool,
                    tc.tile_pool(name="ev2", bufs=4) as ev2_pool,
                ):
                    for br, (tab, src) in enumerate([(am_fe, we),
                                                     (am_fo, wd)]):
                        for up in range(4):  # 512 half-u's per panel
                            am = am_pool.tile([128, _KT2 * _RPC], bf16,
                                              tag="am")
                            nc.sync.dma_start(
                                out=am[:].rearrange("p (t q) -> p t q",
                                                    t=_KT2),
                                in_=tab[:, up * _RPC:(up + 1) * _RPC]
                                .rearrange("(t p) q -> p t q", p=128))
                            for sub in range(RT):
                                e = up * RT + sub  # u-tile index 0..15
                                ps = ps2_pool.tile([128, _RPC], fp32,
                                                   tag="ps2")
                                for k in range(_KT2):
                                    nc.tensor.matmul(
                                        ps[:],
                                        am[:, k * _RPC + sub * 128:
                                               k * _RPC + sub * 128 + 128],
                                        src[:, k * _RPC:(k + 1) * _RPC],
                                        start=(k == 0), stop=(k == _KT2 - 1))
                                ev = ev2_pool.tile([128, _RPC], bf16,
                                                   tag="ev2")
                                nc.vector.tensor_copy(ev[:], ps[:])
                                # u = 2*(128e + i) + br ; block j = e//2,
                                # rows = 256*(e%2) + 2*i + br
                                csv = c_send[:].rearrange(
                                    "j (r2 par) c -> j r2 par c", par=2)
                                nc.sync.dma_start(
                                    out=csv[e // 2,
                                            (e % 2) * 128:(e % 2) * 128 + 128,
                                            br, :],
                                    in_=ev[:])

            # ============ exchange 2 ============
            nc.gpsimd.collective_compute(
                "AllToAll",
                mybir.AluOpType.bypass,
                ins=[c_send[:]],
                outs=[c_recv[:]],
                replica_groups=[list(range(_NCORES))],
            )

            # ============ final: un-permute v, convert to fp32 ============
            with (
                tc.tile_pool(name="cr", bufs=2) as cr_pool,
                tc.tile_pool(name="co", bufs=2) as co_pool,
            ):
                for rt in range(RT):
                    cr = cr_pool.tile([128, _NCORES * _RPC], bf16, tag="cr")
                    nc.sync.dma_start(
                        out=cr[:].rearrange("p (m pc) -> p m pc", m=_NCORES),
                        in_=c_recv[:, rt * 128:(rt + 1) * 128, :]
                        .rearrange("m p pc -> p m pc"))
                    co = co_pool.tile([128, _SZ], fp32, tag="co")
                    # cout[p, 512m + 2c + par] = cr[p, m, par*256 + c]
                    nc.vector.tensor_copy(
                        co[:].rearrange("p (m c par) -> p m par c",
                                        par=2, c=256),
                        cr[:].rearrange("p (m par c) -> p m par c",
                                        par=2, c=256))
                    nc.sync.dma_start(
                        out=cout[rt * 128:(rt + 1) * 128, :], in_=co[:])

    nc.compile()
    return nc


# --------------------------------------------------------------------------
# PJRT SPMD runner (compile once, run many; single jit per call)
# --------------------------------------------------------------------------
def _build_runner(nc, n_cores):
    import jax
    from jax.sharding import Mesh, NamedSharding, PartitionSpec
    from jax.experimental.shard_map import shard_map
    import concourse.mybir as mybir
    from concourse import bass2jax
    from concourse.bass2jax import _bass_exec_p, partition_id_tensor

    bass2jax.install_neuronx_cc_hook()
    partition_name = (nc.partition_id_tensor.name
                      if nc.partition_id_tensor else None)

    in_names, out_names, out_avals = [], [], []
    for alloc in nc.m.functions[0].allocations:
        if not isinstance(alloc, mybir.MemoryLocationSet):
            continue
        name = alloc.memorylocations[0].name
        if alloc.kind == "ExternalInput":
            if name != partition_name:
                in_names.append(name)
        elif alloc.kind == "ExternalOutput":
            shape = tuple(alloc.tensor_shape)
            dtype = mybir.dt.np(alloc.dtype)
            out_names.append(name)
            out_avals.append(jax.core.ShapedArray(shape, dtype))
    in_names_all = list(in_names) + out_names
    if partition_name is not None:
        in_names_all = in_names_all + [partition_name]

    assert in_names[0] == "x_in", in_names
    assert out_names == ["cout"], out_names
    n_rep = len(in_names) - 1  # replicated table inputs

    devices = jax.devices()[:n_cores]
    mesh = Mesh(np.asarray(devices), ("core",))
    shard_rows = NamedSharding(mesh, PartitionSpec("core"))
    shard_rep = NamedSharding(mesh, PartitionSpec())

    def _body(*args):
        operands = list(args)
        if partition_name is not None:
            operands.append(partition_id_tensor())
        outs = _bass_exec_p.bind(
            *operands,
            out_avals=tuple(out_avals),
            in_names=tuple(in_names_all),
            out_names=tuple(out_names),
            lowering_input_output_aliases=(),
            sim_require_finite=True,
            sim_require_nnan=True,
            nc=nc,
        )
        return outs[0]

    n_outs = len(out_avals)
    sharded = jax.jit(
        shard_map(_body, mesh=mesh,
                  in_specs=(PartitionSpec("core"),)
                  + (PartitionSpec(),) * n_rep
                  + (PartitionSpec("core"),) * n_outs,
                  out_specs=PartitionSpec("core"),
                  check_rep=False))

    def _zeros():
        # cached, undonated: the kernel fully writes cout every call, so
        # the initial content of the bound output buffers never matters.
        if "zeros" not in _state:
            z = tuple(
                jax.device_put(
                    np.zeros((n_cores * a.shape[0], *a.shape[1:]), a.dtype),
                    shard_rows)
                for a in out_avals)
            jax.block_until_ready(z)
            _state["zeros"] = z
        return _state["zeros"]

    def run(x_np, tables):
        xd = jax.device_put(x_np, shard_rows)
        return sharded(xd, *tables, *_zeros())

    run.in_names = in_names
    run.shard_rep = shard_rep
    return run


# --------------------------------------------------------------------------
# host-side tables (bf16 folded; device-cached)
# --------------------------------------------------------------------------
def _tables_np(expkM, expkN):
    import ml_dtypes
    n = _SZ
    h = n // 2
    i = np.arange(h)
    v = np.arange(n)
    # pinv[2i] = i  ->  folded row i corresponds to original column j = 2i
    idx = (i[:, None] * v[None, :]) % n  # (h, n)
    lut_c = np.cos((2.0 * np.pi / n) * np.arange(n))
    lut_s = np.sin((2.0 * np.pi / n) * np.arange(n))
    Cp = lut_c[idx]
    Sp = lut_s[idx]
    eNr = expkN[:, 0].astype(np.float64)
    eNi = expkN[:, 1].astype(np.float64)
    eMr = expkM[:, 0].astype(np.float64)
    eMi = expkM[:, 1].astype(np.float64)
    annT_f = 2.0 * (Cp * eNr[None, :] + Sp * eNi[None, :])
    amT_f = 0.5 * (Cp * eMr[None, :] + Sp * eMi[None, :])
    bf = ml_dtypes.bfloat16
    return (np.ascontiguousarray(annT_f[:, 0::2]).astype(bf),
            np.ascontiguousarray(annT_f[:, 1::2]).astype(bf),
            np.ascontiguousarray(amT_f[:, 0::2]).astype(bf),
            np.ascontiguousarray(amT_f[:, 1::2]).astype(bf),
            _fold_mats())


def _tables(expkM, expkN, run):
    import jax
    key = (expkM.tobytes(), expkN.tobytes())
    cached = _state.get("tables")
    if cached is not None and cached[0] == key:
        return cached[1]
    tabs = _tables_np(expkM, expkN)
    named = dict(zip(["ann_fe", "ann_fo", "am_fe", "am_fo", "fold_m"], tabs))
    devs = tuple(jax.device_put(named[nm], run.shard_rep)
                 for nm in run.in_names[1:])
    jax.block_until_ready(devs)
    _state["tables"] = (key, devs)
    return devs


def kernel(x, expkM, expkN, M, N):
    x = np.asarray(x, dtype=np.float32)
    expkM = np.asarray(expkM, dtype=np.float32)
    expkN = np.asarray(expkN, dtype=np.float32)
    assert x.shape == (_SZ, _SZ)

    if "run" not in _state:
        nc = _build_bass()
        _state["nc"] = nc
        _state["run"] = _build_runner(nc, _NCORES)
    run = _state["run"]
    tabs = _tables(expkM, expkN, run)
    out = run(x, tabs)
    return np.asarray(out)


# revision 14
# speedup vs baseline: 1923.8228x; 1.0216x over previous
"""2D DCT-II (4096x4096) on 8 Trainium2 NeuronCores (axon/PJRT SPMD).

Math: C = A_M @ x @ A_N^T with A tables folding the Makhoul permutation and
expk twiddles (annT[j, v] = A_N[v, j], amT[g, u] = A_M[u, g]).

Fold trick (halves both matmuls): rows of annT pair up as
  annT[pm[i+n/2], v] = (-1)^v * annT[pm[i], v],   pm[i] = 2i, pm[i+n/2] = 4095-2i
so with e_i = x[:, 2i] + x[:, 4095-2i], d_i = x[:, 2i] - x[:, 4095-2i]:
  W[:, even v] = e @ annT_f[:, even v],  W[:, odd v] = d @ annT_f[:, odd v]
where annT_f = annT[0::2] is the 2048-row folded table. Same structure for
amT in the second (row-transform) phase. Folds are computed on TensorE with
six constant 128x128 comb matrices (contraction pairs live on partitions).

Device schedule (core k of 8, all matmuls bf16 into fp32 PSUM):
  phase 0: DMA x[rows_k], TensorE-transpose to xw[c-part, (ct, r)],
           fold into xe/xd [i-part, (t, r)] (i < 2048).
  phase 1: W_k rows: even-v from xe @ ann_fe, odd-v from xd @ ann_fo;
           blocks stored v-permuted: w_send[j, r, 0:256]=evens, [256:512]=odds.
  AllToAll #1 (bf16); phase 2: fold W rows g (same combs) -> we/wd;
           C[even u, cols_k] = am_fe.T @ we, odd u from am_fo.T @ wd,
           written to c_send[j] with stride-2 rows (true row order).
  AllToAll #2 (bf16); final: convert bf16->fp32 and un-permute v within
           each 512-block: cout[r, 512m+2c(+1)] = c_recv[m, r, (par*256)+c].
Host: no transposes/concats; x device_put row-sharded; output of shard_map
IS the full (4096,4096) C. Tables device-cached (32MB bf16 total).
"""
import numpy as np

_NCORES = 8
_SZ = 4096
_RPC = _SZ // _NCORES  # 512 rows/cols per core
_KT = _SZ // 128       # 32 c-tiles (unfolded)
_KT2 = _KT // 2        # 16 folded contraction tiles

_state = {}


def _fold_mats():
    """Six 128x128 bf16 comb matrices for the fold matmuls.
    e-tile t chain: M0@xw[2t] + M1@xw[2t+1] + M2@xw[31-2t] + M3@xw[30-2t]
    d-tile t chain: M0, M1, -M2, -M3."""
    import ml_dtypes
    M = np.zeros((6, 128, 128), np.float32)
    for i in range(64):
        M[0, 2 * i, i] = 1.0            # g' = 2i', i' < 64
        M[1, 2 * i, 64 + i] = 1.0       # g' = 2(i'-64), i' >= 64
    for i in range(64):
        M[2, 127 - 2 * i, i] = 1.0      # mirror, i' < 64
        M[3, 255 - 2 * (64 + i) - 0, 64 + i] = 1.0  # g' = 255-2i', i' >= 64
    M[4] = -M[2]
    M[5] = -M[3]
    return M.astype(ml_dtypes.bfloat16)


# --------------------------------------------------------------------------
# Bass kernel
# --------------------------------------------------------------------------
def _build_bass():
    import concourse.bacc as bacc
    import concourse.mybir as mybir
    from concourse.tile import TileContext
    from concourse import masks

    fp32 = mybir.dt.float32
    bf16 = mybir.dt.bfloat16
    nc = bacc.Bacc("TRN2", target_bir_lowering=False, debug=False,
                   num_devices=_NCORES)
    x_in = nc.declare_dram_parameter("x_in", [_RPC, _SZ], fp32,
                                     isOutput=False)
    ann_fe = nc.declare_dram_parameter("ann_fe", [_SZ // 2, _SZ // 2], bf16,
                                       isOutput=False)
    ann_fo = nc.declare_dram_parameter("ann_fo", [_SZ // 2, _SZ // 2], bf16,
                                       isOutput=False)
    am_fe = nc.declare_dram_parameter("am_fe", [_SZ // 2, _SZ // 2], bf16,
                                      isOutput=False)
    am_fo = nc.declare_dram_parameter("am_fo", [_SZ // 2, _SZ // 2], bf16,
                                      isOutput=False)
    fold_m = nc.declare_dram_parameter("fold_m", [6, 128, 128], bf16,
                                       isOutput=False)
    cout = nc.declare_dram_parameter("cout", [_RPC, _SZ], fp32,
                                     isOutput=True)

    w_send = [nc.dram_tensor(f"w_send{h}", [_NCORES, _RPC, 256], bf16)
              for h in range(2)]
    w_recv = [nc.dram_tensor(f"w_recv{h}", [_NCORES, _RPC, 256], bf16)
              for h in range(2)]
    c_send = [nc.dram_tensor(f"c_send{h}", [_NCORES, _RPC, 256], bf16)
              for h in range(2)]
    c_recv = [nc.dram_tensor(f"c_recv{h}", [_NCORES, _RPC, 256], bf16)
              for h in range(2)]

    RT = _RPC // 128  # 4 row tiles per core

    def fold_chain(ps, fm, neg, src, t, width):
        """Accumulate the 4-matmul fold chain for tile t into psum ps.
        fm: SBUF tile [128, 6*128] holding combs column-blocked."""
        if neg:  # d-chain: negated mirror combs
            pairs = [(0, 2 * t), (1, 2 * t + 1), (4, 31 - 2 * t),
                     (5, 30 - 2 * t)]
        else:
            pairs = [(0, 2 * t), (1, 2 * t + 1), (2, 31 - 2 * t),
                     (3, 30 - 2 * t)]
        for n_i, (mi, gt) in enumerate(pairs):
            nc.tensor.matmul(
                ps[:], fm[:, mi * 128:(mi + 1) * 128],
                src[:, gt * width:(gt + 1) * width],
                start=(n_i == 0), stop=(n_i == 3))

    with TileContext(nc) as tc:
        with (
            tc.tile_pool(name="fm", bufs=1) as fm_pool,
            tc.tile_pool(name="pre", bufs=1) as pre_pool,
            tc.tile_pool(name="psfold", bufs=2, space="PSUM") as psfold_pool,
        ):
            fmt = fm_pool.tile([128, 6 * 128], bf16)
            nc.sync.dma_start(
                out=fmt[:].rearrange("p (m c) -> p m c", m=6),
                in_=fold_m[:].rearrange("m p c -> p m c"))
            fm = fmt[:]
            # phase-2 half-a working set, preallocated so its loads/folds
            # overlap phase 1 (no pool-swap serialization):
            wrt = [pre_pool.tile([128, _KT * 256], bf16, name=f"wrt{h}")
                   for h in range(2)]
            wet = pre_pool.tile([128, _KT2 * 256], bf16)
            wdt = pre_pool.tile([128, _KT2 * 256], bf16)
            we2t = pre_pool.tile([128, 8 * 256], bf16)
            wd2t = pre_pool.tile([128, 8 * 256], bf16)
            amee = [pre_pool.tile([128, 8 * _RPC], bf16, name=f"amee{u}")
                    for u in range(2)]
            ameo = [pre_pool.tile([128, 8 * _RPC], bf16, name=f"ameo{u}")
                    for u in range(2)]

            # ============ phase 0: transpose + fold x ============
            with (
                tc.tile_pool(name="ident", bufs=1) as ident_pool,
                tc.tile_pool(name="xev", bufs=1) as xe_pool,
                tc.tile_pool(name="xw", bufs=1) as xw_pool,
            ):
                identf = ident_pool.tile([128, 128], fp32)
                masks.make_identity(nc, identf[:])
                # xe: [i-part, (t, r)] folded sums; xd: diffs  (16 KB/part ea)
                xe = xe_pool.tile([128, _KT2 * _RPC], bf16)
                xd = xe_pool.tile([128, _KT2 * _RPC], bf16)
                xw = xw_pool.tile([128, _KT * _RPC], bf16)  # 32 KB/part

                with (
                    tc.tile_pool(name="xf", bufs=2) as xf_pool,
                    tc.tile_pool(name="pst", bufs=3, space="PSUM") as pst_pool,
                    tc.tile_pool(name="psf", bufs=3, space="PSUM") as psf_pool,
                ):
                    for rt in range(RT):
                        nc.sync.dma_start(
                            out=xf[:, rt * _SZ:(rt + 1) * _SZ],
                            in_=x_in[rt * 128:(rt + 1) * 128, :])
                    for ct in range(_KT):
                        ps = pst_pool.tile([128, _RPC], fp32, tag="pst")
                        for rt in range(RT):
                            nc.tensor.transpose(
                                ps[:, rt * 128:(rt + 1) * 128],
                                xf[:, rt * _SZ + ct * 128:
                                      rt * _SZ + ct * 128 + 128],
                                identf[:])
                        nc.vector.tensor_copy(
                            xw[:, ct * _RPC:(ct + 1) * _RPC], ps[:])
                    for t in range(_KT2):
                        pe = psf_pool.tile([128, _RPC], fp32, tag="psf")
                        fold_chain(pe, fm, False, xw, t, _RPC)
                        nc.vector.tensor_copy(
                            xe[:, t * _RPC:(t + 1) * _RPC], pe[:])
                        pd = psf_pool.tile([128, _RPC], fp32, tag="psf")
                        fold_chain(pd, fm, True, xw, t, _RPC)
                        nc.vector.tensor_copy(
                            xd[:, t * _RPC:(t + 1) * _RPC], pd[:])

                # ============ phase 1: W = x @ A_N^T (folded) ============
                with (
                    tc.tile_pool(name="an", bufs=3) as an_pool,
                    tc.tile_pool(name="ps1", bufs=6, space="PSUM") as ps1_pool,
                    tc.tile_pool(name="ev1", bufs=4) as ev1_pool,
                ):
                    for br, (tab, src) in enumerate(
                            [(ann_fe, xe), (ann_fo, xd)]):
                        for vp in range(4):  # 512 q's per panel
                            an = an_pool.tile([128, _KT2 * _RPC], bf16,
                                              tag="an")
                            nc.sync.dma_start(
                                out=an[:].rearrange("p (t q) -> p t q",
                                                    t=_KT2),
                                in_=tab[:, vp * _RPC:(vp + 1) * _RPC]
                                .rearrange("(t p) q -> p t q", p=128))
                            for rt in range(RT):
                                ps = ps1_pool.tile([128, _RPC], fp32,
                                                   tag="ps")
                                for k in range(_KT2):
                                    nc.tensor.matmul(
                                        ps[:],
                                        src[:, k * _RPC + rt * 128:
                                               k * _RPC + rt * 128 + 128],
                                        an[:, k * _RPC:(k + 1) * _RPC],
                                        start=(k == 0), stop=(k == _KT2 - 1))
                                ev = ev1_pool.tile([128, _RPC], bf16,
                                                   tag="ev")
                                nc.vector.tensor_copy(ev[:], ps[:])
                                c0 = br * 256  # evens 0:256, odds 256:512
                                nc.sync.dma_start(
                                    out=w_send[2 * vp,
                                               rt * 128:(rt + 1) * 128,
                                               c0:c0 + 256],
                                    in_=ev[:, 0:256])
                                nc.sync.dma_start(
                                    out=w_send[2 * vp + 1,
                                               rt * 128:(rt + 1) * 128,
                                               c0:c0 + 256],
                                    in_=ev[:, 256:512])

            # ============ exchange 1 ============
            nc.gpsimd.collective_compute(
                "AllToAll",
                mybir.AluOpType.bypass,
                ins=[w_send[:]],
                outs=[w_recv[:]],
                replica_groups=[list(range(_NCORES))],
            )

            # ============ phase 2: C = A_M @ W (folded) ============
            with (
                tc.tile_pool(name="wev", bufs=1) as we_pool,
            ):
                # we: [i-part, (t, v)] folded sums of W rows; wd: diffs
                we = we_pool.tile([128, _KT2 * _RPC], bf16)
                wd = we_pool.tile([128, _KT2 * _RPC], bf16)
                with (
                    tc.tile_pool(name="wr", bufs=1) as wr_pool,
                    tc.tile_pool(name="psf2", bufs=4, space="PSUM")
                    as psf2_pool,
                ):
                    wr = wr_pool.tile([128, _KT * _RPC], bf16)  # 32 KB/part
                    nc.sync.dma_start(
                        out=wr[:].rearrange("p (m s v) -> p m s v",
                                            m=_NCORES, s=4),
                        in_=w_recv[:].rearrange("m (s p) v -> p m s v",
                                                p=128))
                    for t in range(_KT2):
                        pe = psf2_pool.tile([128, _RPC], fp32, tag="psf2")
                        fold_chain(pe, fm, False, wr, t, _RPC)
                        nc.vector.tensor_copy(
                            we[:, t * _RPC:(t + 1) * _RPC], pe[:])
                        pd = psf2_pool.tile([128, _RPC], fp32, tag="psf2")
                        fold_chain(pd, fm, True, wr, t, _RPC)
                        nc.vector.tensor_copy(
                            wd[:, t * _RPC:(t + 1) * _RPC], pd[:])

                with (
                    tc.tile_pool(name="am", bufs=3) as am_pool,
                    tc.tile_pool(name="ps2", bufs=8, space="PSUM") as ps2_pool,
                    tc.tile_pool(name="ev2", bufs=4) as ev2_pool,
                ):
                    for br, (tab, src) in enumerate([(am_fe, we),
                                                     (am_fo, wd)]):
                        for up in range(4):  # 512 half-u's per panel
                            am = am_pool.tile([128, _KT2 * _RPC], bf16,
                                              tag="am")
                            nc.sync.dma_start(
                                out=am[:].rearrange("p (t q) -> p t q",
                                                    t=_KT2),
                                in_=tab[:, up * _RPC:(up + 1) * _RPC]
                                .rearrange("(t p) q -> p t q", p=128))
                            for sub in range(RT):
                                e = up * RT + sub  # u-tile index 0..15
                                ps = ps2_pool.tile([128, _RPC], fp32,
                                                   tag="ps2")
                                for k in range(_KT2):
                                    nc.tensor.matmul(
                                        ps[:],
                                        am[:, k * _RPC + sub * 128:
                                               k * _RPC + sub * 128 + 128],
                                        src[:, k * _RPC:(k + 1) * _RPC],
                                        start=(k == 0), stop=(k == _KT2 - 1))
                                ev = ev2_pool.tile([128, _RPC], bf16,
                                                   tag="ev2")
                                nc.vector.tensor_copy(ev[:], ps[:])
                                # u = 2*(128e + i) + br ; block j = e//2,
                                # rows = 256*(e%2) + 2*i + br
                                csv = c_send[:].rearrange(
                                    "j (r2 par) c -> j r2 par c", par=2)
                                nc.sync.dma_start(
                                    out=csv[e // 2,
                                            (e % 2) * 128:(e % 2) * 128 + 128,
                                            br, :],
                                    in_=ev[:])

            # ============ exchange 2 ============
            nc.gpsimd.collective_compute(
                "AllToAll",
                mybir.AluOpType.bypass,
                ins=[c_send[:]],
                outs=[c_recv[:]],
                replica_groups=[list(range(_NCORES))],
            )

            # ============ final: un-permute v, convert to fp32 ============
            with (
                tc.tile_pool(name="cr", bufs=2) as cr_pool,
                tc.tile_pool(name="co", bufs=2) as co_pool,
            ):
                for rt in range(RT):
                    cr = cr_pool.tile([128, _NCORES * _RPC], bf16, tag="cr")
                    nc.sync.dma_start(
                        out=cr[:].rearrange("p (m pc) -> p m pc", m=_NCORES),
                        in_=c_recv[:, rt * 128:(rt + 1) * 128, :]
                        .rearrange("m p pc -> p m pc"))
                    co = co_pool.tile([128, _SZ], fp32, tag="co")
                    # cout[p, 512m + 2c + par] = cr[p, m, par*256 + c]
                    nc.vector.tensor_copy(
                        co[:].rearrange("p (m c par) -> p m par c",
                                        par=2, c=256),
                        cr[:].rearrange("p (m par c) -> p m par c",
                                        par=2, c=256))
                    nc.sync.dma_start(
                        out=cout[rt * 128:(rt + 1) * 128, :], in_=co[:])

    nc.compile()
    return nc


# --------------------------------------------------------------------------
# PJRT SPMD runner (compile once, run many; single jit per call)
# --------------------------------------------------------------------------
def _build_runner(nc, n_cores):
    import jax
    from jax.sharding import Mesh, NamedSharding, PartitionSpec
    from jax.experimental.shard_map import shard_map
    import concourse.mybir as mybir
    from concourse import bass2jax
    from concourse.bass2jax import _bass_exec_p, partition_id_tensor

    bass2jax.install_neuronx_cc_hook()
    partition_name = (nc.partition_id_tensor.name
                      if nc.partition_id_tensor else None)

    in_names, out_names, out_avals = [], [], []
    for alloc in nc.m.functions[0].allocations:
        if not isinstance(alloc, mybir.MemoryLocationSet):
            continue
        name = alloc.memorylocations[0].name
        if alloc.kind == "ExternalInput":
            if name != partition_name:
                in_names.append(name)
        elif alloc.kind == "ExternalOutput":
            shape = tuple(alloc.tensor_shape)
            dtype = mybir.dt.np(alloc.dtype)
            out_names.append(name)
            out_avals.append(jax.core.ShapedArray(shape, dtype))
    in_names_all = list(in_names) + out_names
    if partition_name is not None:
        in_names_all = in_names_all + [partition_name]

    assert in_names[0] == "x_in", in_names
    assert out_names == ["cout"], out_names
    n_rep = len(in_names) - 1  # replicated table inputs

    devices = jax.devices()[:n_cores]
    mesh = Mesh(np.asarray(devices), ("core",))
    shard_rows = NamedSharding(mesh, PartitionSpec("core"))
    shard_rep = NamedSharding(mesh, PartitionSpec())

    def _body(*args):
        operands = list(args)
        if partition_name is not None:
            operands.append(partition_id_tensor())
        outs = _bass_exec_p.bind(
            *operands,
            out_avals=tuple(out_avals),
            in_names=tuple(in_names_all),
            out_names=tuple(out_names),
            lowering_input_output_aliases=(),
            sim_require_finite=True,
            sim_require_nnan=True,
            nc=nc,
        )
        return outs[0]

    n_outs = len(out_avals)
    sharded = jax.jit(
        shard_map(_body, mesh=mesh,
                  in_specs=(PartitionSpec("core"),)
                  + (PartitionSpec(),) * n_rep
                  + (PartitionSpec("core"),) * n_outs,
                  out_specs=PartitionSpec("core"),
                  check_rep=False))

    def _zeros():
        # cached, undonated: the kernel fully writes cout every call, so
        # the initial content of the bound output buffers never matters.
        if "zeros" not in _state:
            z = tuple(
                jax.device_put(
                    np.zeros((n_cores * a.shape[0], *a.shape[1:]), a.dtype),
                    shard_rows)
                for a in out_avals)
            jax.block_until_ready(z)
            _state["zeros"] = z
        return _state["zeros"]

    def run(x_np, tables):
        xd = jax.device_put(x_np, shard_rows)
        return sharded(xd, *tables, *_zeros())

    run.in_names = in_names
    run.shard_rep = shard_rep
    return run


# --------------------------------------------------------------------------
# host-side tables (bf16 folded; device-cached)
# --------------------------------------------------------------------------
def _tables_np(expkM, expkN):
    import ml_dtypes
    n = _SZ
    h = n // 2
    i = np.arange(h)
    v = np.arange(n)
    # pinv[2i] = i  ->  folded row i corresponds to original column j = 2i
    idx = (i[:, None] * v[None, :]) % n  # (h, n)
    lut_c = np.cos((2.0 * np.pi / n) * np.arange(n))
    lut_s = np.sin((2.0 * np.pi / n) * np.arange(n))
    Cp = lut_c[idx]
    Sp = lut_s[idx]
    eNr = expkN[:, 0].astype(np.float64)
    eNi = expkN[:, 1].astype(np.float64)
    eMr = expkM[:, 0].astype(np.float64)
    eMi = expkM[:, 1].astype(np.float64)
    annT_f = 2.0 * (Cp * eNr[None, :] + Sp * eNi[None, :])
    amT_f = 0.5 * (Cp * eMr[None, :] + Sp * eMi[None, :])
    bf = ml_dtypes.bfloat16
    return (np.ascontiguousarray(annT_f[:, 0::2]).astype(bf),
            np.ascontiguousarray(annT_f[:, 1::2]).astype(bf),
            np.ascontiguousarray(amT_f[:, 0::2]).astype(bf),
            np.ascontiguousarray(amT_f[:, 1::2]).astype(bf),
            _fold_mats())


def _tables(expkM, expkN, run):
    import jax
    key = (expkM.tobytes(), expkN.tobytes())
    cached = _state.get("tables")
    if cached is not None and cached[0] == key:
        return cached[1]
    tabs = _tables_np(expkM, expkN)
    named = dict(zip(["ann_fe", "ann_fo", "am_fe", "am_fo", "fold_m"], tabs))
    devs = tuple(jax.device_put(named[nm], run.shard_rep)
                 for nm in run.in_names[1:])
    jax.block_until_ready(devs)
    _state["tables"] = (key, devs)
    return devs


def kernel(x, expkM, expkN, M, N):
    x = np.asarray(x, dtype=np.float32)
    expkM = np.asarray(expkM, dtype=np.float32)
    expkN = np.asarray(expkN, dtype=np.float32)
    assert x.shape == (_SZ, _SZ)

    if "run" not in _state:
        nc = _build_bass()
        _state["nc"] = nc
        _state["run"] = _build_runner(nc, _NCORES)
    run = _state["run"]
    tabs = _tables(expkM, expkN, run)
    out = run(x, tabs)
    return np.asarray(out)
